# revision 1
# baseline (speedup 1.0000x reference)
"""Multi-head attention (EMB=512, HEADS=8, x:(4,2048,512)) on 8 Trainium2 cores.

Sharding: zero-collective split — core c handles batch c//2, query rows
(c%2)*1024..(c%2+1)*1024, ALL heads.  K/V projections for the full batch are
computed redundantly on the 2 cores sharing a batch (16% extra PE work, but no
collectives at all).

Device-side dataflow (per core, everything SBUF-resident):
  xT (host-transposed)           [512, 2048]   keys reordered so queries first
  Q^T = WqT.T @ xT  (+bq)        [512, 1024]   feature-major
  K^T = WkT.T @ xT  (+bk)        [512, 2048]   feature-major
  V~  = xT.T @ WvT  (+bv, ones)  [2048, 8*65]  token-major, per-head ones col
  S^T = K^T_h.T @ Q^T_h          [2048, 512]   per (head, query-chunk)
  P^T = exp(S^T / sqrt(512))     (ScalarE, fused drain from PSUM)
  outT~ = V~_h.T @ P^T           [65, 512]     row 64 = softmax denominator
  R = sel.T @ recip(sums)        partition-broadcast of 1/denominator via PE
  outT = outT~ * R               normalized, feature-major
  y = outT.T @ WoT (+bo)         [1024, 512]   token-major, DMA out
"""

import sys
import os

for _p in ("/opt/trn_rl_repo", "/root/.axon_site/_ro/trn_rl_repo"):
    if os.path.isdir(_p) and _p not in sys.path:
        sys.path.append(_p)

import numpy as np

EMB = 512
HEADS = 8
D = 64  # head dim
B = 4
N = 2048  # keys / tokens per batch
HALF = 1024  # queries per core
P = 128
NCORES = 8
KT4 = EMB // P  # 4 contraction tiles
SCALE = float(1.0 / np.sqrt(np.float32(EMB)))

_CACHE = {}


def _build_program(debug=False):
    from concourse import bacc
    import concourse.mybir as mybir
    import concourse.tile as tile
    from contextlib import ExitStack

    dt = mybir.dt.float32
    # float32r: same 4-byte storage as fp32, TF32-style single-pass matmul
    # (4x faster than fp32). All matmul-feeding tensors are declared f32r;
    # producers (DMA/DVE/ACT) round on write, everything else reads it as fp32.
    f32r = mybir.dt.float32r
    bf16 = mybir.dt.bfloat16
    nc = bacc.Bacc("TRN2", target_bir_lowering=False)

    xT_d = nc.dram_tensor("xT", [KT4, P, N], f32r, kind="ExternalInput")
    wq_d = nc.dram_tensor("wq", [KT4, P, EMB], f32r, kind="ExternalInput")
    wk_d = nc.dram_tensor("wk", [KT4, P, EMB], f32r, kind="ExternalInput")
    wv_d = nc.dram_tensor("wv", [KT4, P, EMB], f32r, kind="ExternalInput")
    wo_d = nc.dram_tensor("wo", [KT4, P, EMB], f32r, kind="ExternalInput")
    bq_d = nc.dram_tensor("bq2", [P, KT4], dt, kind="ExternalInput")
    bk_d = nc.dram_tensor("bk2", [P, KT4], dt, kind="ExternalInput")
    bvr_d = nc.dram_tensor("bvr", [P, EMB], dt, kind="ExternalInput")
    bor_d = nc.dram_tensor("bor", [P, EMB], dt, kind="ExternalInput")
    sel_d = nc.dram_tensor("sel", [HEADS, KT4, P], f32r, kind="ExternalInput")
    y_d = nc.dram_tensor("y", [HALF, EMB], dt, kind="ExternalOutput")
    if debug:
        dQT = nc.dram_tensor("dQT", [P, KT4, HALF], f32r, kind="ExternalOutput")
        dKT = nc.dram_tensor("dKT", [P, KT4, N], f32r, kind="ExternalOutput")
        dVb = nc.dram_tensor("dVb", [P, 16, HEADS, D + 8], bf16, kind="ExternalOutput")
        dsums = nc.dram_tensor("dsums", [HEADS, HALF], dt, kind="ExternalOutput")
        doutT = nc.dram_tensor("doutT", [P, KT4, HALF], f32r, kind="ExternalOutput")
        dPT = nc.dram_tensor("dPT", [P, 1024], bf16, kind="ExternalOutput")
        drsum = nc.dram_tensor("drsum", [HEADS, HALF], dt, kind="ExternalOutput")
        doutU = nc.dram_tensor("doutU", [P, KT4, HALF], f32r, kind="ExternalOutput")

    Exp = mybir.ActivationFunctionType.Exp
    mult = mybir.AluOpType.mult
    add = mybir.AluOpType.add

    with tile.TileContext(nc) as tc, ExitStack() as ctx:
        # "big" slots (8KB/part): 4 x xT during projections, then recycled as
        # P^T chunks during attention.
        big = ctx.enter_context(tc.tile_pool(name="big", bufs=4))
        ptp = ctx.enter_context(tc.tile_pool(name="ptp", bufs=6))
        wp = ctx.enter_context(tc.tile_pool(name="wp", bufs=1))
        pers = ctx.enter_context(tc.tile_pool(name="pers", bufs=1))
        yp = ctx.enter_context(tc.tile_pool(name="yp", bufs=2))
        # PSUM: tag "s" 3 x [128,1024] slots (6 banks) + tag "pv" 2 x 1 bank
        ps = ctx.enter_context(tc.tile_pool(name="ps", bufs=3, space="PSUM"))

        # ---- input loads ----
        xt = []
        for kt in range(KT4):
            t = big.tile([P, N], f32r, name=f"xt{kt}", tag="big")
            nc.sync.dma_start(t[:], xT_d[kt])
            xt.append(t)

        def load_w(dram, nm):
            t = wp.tile([P, KT4, EMB], f32r, name=nm, tag=nm)
            for kt in range(KT4):
                nc.sync.dma_start(t[:, kt], dram[kt])
            return t

        wq_s = load_w(wq_d, "wqs")
        wk_s = load_w(wk_d, "wks")
        wv_s = load_w(wv_d, "wvs")
        wo_s = load_w(wo_d, "wos")
        bq_s = pers.tile([P, KT4], dt, name="bqs")
        nc.sync.dma_start(bq_s[:], bq_d[:])
        bk_s = pers.tile([P, KT4], dt, name="bks")
        nc.sync.dma_start(bk_s[:], bk_d[:])
        bvr_s = pers.tile([P, HEADS, D], dt, name="bvrs")
        nc.sync.dma_start(bvr_s[:], bvr_d.ap().rearrange("p (h d) -> p h d", d=D))
        bor_s = pers.tile([P, EMB], dt, name="bors")
        nc.sync.dma_start(bor_s[:], bor_d[:])
        sel_s = pers.tile([HEADS, KT4, P], f32r, name="sels")
        nc.sync.dma_start(sel_s[:], sel_d[:])

        # ---- persistent intermediates ----
        QT = pers.tile([P, KT4, HALF], f32r, name="QT")
        KTt = pers.tile([P, KT4, N], f32r, name="KTt")
        Vb = pers.tile([P, 16, HEADS, D + 8], bf16, name="Vb")
        outT = pers.tile([P, KT4, HALF], f32r, name="outT")
        sums = pers.tile([64 + HEADS, HALF], dt, name="sums")
        sums0 = pers.tile([HEADS, HALF], dt, name="sums0")
        rsum = pers.tile([HEADS, HALF], dt, name="rsum")

        # per-head one-hot indicator columns: PV lands head h's softmax
        # denominator on PSUM partition 64+h (32-aligned drains, distinct rows)
        nc.vector.memset(Vb[:, :, :, D:D + 8], 0.0)
        for h in range(HEADS):
            nc.vector.memset(Vb[:, :, h, D + h], 1.0)
        nc.vector.memset(sums[64:64 + HEADS, :], 0.0)

        def emit_q(jt):
            for c in range(2):
                pq = ps.tile([P, 512], dt, tag="s", name=f"pq{jt}{c}")
                for kt in range(KT4):
                    nc.tensor.matmul(
                        pq[:],
                        lhsT=wq_s[:, kt, jt * P:(jt + 1) * P],
                        rhs=xt[kt][:, c * 512:(c + 1) * 512],
                        start=kt == 0,
                        stop=kt == KT4 - 1,
                    )
                nc.vector.tensor_scalar_add(
                    QT[:, jt, c * 512:(c + 1) * 512], pq[:], bq_s[:, jt:jt + 1]
                )

        def emit_k(jt):
            for c in range(4):
                pk = ps.tile([P, 512], dt, tag="s", name=f"pk{jt}{c}")
                for kt in range(KT4):
                    nc.tensor.matmul(
                        pk[:],
                        lhsT=wk_s[:, kt, jt * P:(jt + 1) * P],
                        rhs=xt[kt][:, c * 512:(c + 1) * 512],
                        start=kt == 0,
                        stop=kt == KT4 - 1,
                    )
                nc.vector.tensor_scalar_add(
                    KTt[:, jt, c * 512:(c + 1) * 512], pk[:], bk_s[:, jt:jt + 1]
                )

        def emit_v():
            for t in range(16):
                pv = ps.tile([P, 512], dt, tag="s", name=f"pvv{t}")
                for kt in range(KT4):
                    nc.tensor.matmul(
                        pv[:],
                        lhsT=xt[kt][:, t * P:(t + 1) * P],
                        rhs=wv_s[:, kt, :],
                        start=kt == 0,
                        stop=kt == KT4 - 1,
                    )
                nc.vector.tensor_tensor(
                    Vb[:, t, :, 0:D],
                    pv.rearrange("p (h d) -> p h d", d=D),
                    bvr_s[:],
                    add,
                )

        def emit_attn(hp):
            hA, hB = 2 * hp, 2 * hp + 1
            jt = hp  # feature tile holding this head pair
            for c in range(2):
                pvA = ps.tile([D + 8, 512], dt, tag="pv", bufs=2, name=f"pvA{hp}{c}")
                pvB = ps.tile([D + 8, 512], dt, tag="pv", bufs=2, name=f"pvB{hp}{c}")
                for g in range(8):  # pair-groups of 2 key-tiles
                    sA = ps.tile([P, 1024], dt, tag="s", name=f"sA{hp}{c}{g}")
                    sB = ps.tile([P, 1024], dt, tag="s", name=f"sB{hp}{c}{g}")
                    for tt in range(2):
                        t = 2 * g + tt
                        # head A on PE rows 0:64, head B on rows 64:128 —
                        # concurrent via row tiling
                        nc.tensor.matmul(
                            sA[:, tt * 512:(tt + 1) * 512],
                            lhsT=KTt[0:D, jt, t * P:(t + 1) * P],
                            rhs=QT[0:D, jt, c * 512:(c + 1) * 512],
                            start=True,
                            stop=True,
                        )
                        nc.tensor.matmul(
                            sB[:, tt * 512:(tt + 1) * 512],
                            lhsT=KTt[D:P, jt, t * P:(t + 1) * P],
                            rhs=QT[D:P, jt, c * 512:(c + 1) * 512],
                            start=True,
                            stop=True,
                        )
                    ptA = ptp.tile([P, 1024], bf16, tag="pt", name=f"ptA{hp}{c}{g}")
                    ptB = ptp.tile([P, 1024], bf16, tag="pt", name=f"ptB{hp}{c}{g}")
                    nc.scalar.activation(ptA[:], sA[:], Exp, scale=SCALE)
                    if debug and hp == 0 and c == 0 and g == 0:
                        nc.sync.dma_start(dPT.ap(), ptA[:])
                    nc.scalar.activation(ptB[:], sB[:], Exp, scale=SCALE)
                    for tt in range(2):
                        t = 2 * g + tt
                        nc.tensor.matmul(
                            pvA[:],
                            lhsT=Vb[:, t, hA, :],
                            rhs=ptA[:, tt * 512:(tt + 1) * 512],
                            start=t == 0,
                            stop=t == 15,
                        )
                        nc.tensor.matmul(
                            pvB[:],
                            lhsT=Vb[:, t, hB, :],
                            rhs=ptB[:, tt * 512:(tt + 1) * 512],
                            start=t == 0,
                            stop=t == 15,
                        )
                for pv_, h in ((pvA, hA), (pvB, hB)):
                    po = (h % 2) * D
                    nc.vector.tensor_copy(
                        outT[po:po + D, h // 2, c * 512:(c + 1) * 512], pv_[0:D, :]
                    )
                    nc.vector.tensor_tensor(
                        sums[64:64 + HEADS, c * 512:(c + 1) * 512],
                        sums[64:64 + HEADS, c * 512:(c + 1) * 512],
                        pv_[D:D + 8, :],
                        add,
                    )

        if debug:
            nc.sync.dma_start(doutU.ap(), outT[:])

        # Interleaved emission: attention for head-pair jt overlaps the
        # projections for jt+1 (proj matmuls fill PE stalls while ScalarE
        # works through the exp stream).
        emit_q(0)
        emit_k(0)
        emit_v()
        for hp in range(4):
            emit_attn(hp)
            if hp + 1 < KT4:
                emit_q(hp + 1)
                emit_k(hp + 1)

        # ---- normalize: outT *= broadcast(1/sums) ----
        # reciprocal_approx_fast is broken at partition base 64 — move to base 0
        nc.vector.tensor_copy(sums0[:], sums[64:64 + HEADS, :])
        nc.vector.reciprocal_approx_fast(rsum[:], sums0[:])
        rsumr = pers.tile([HEADS, HALF], f32r, name="rsumr")
        nc.vector.tensor_copy(rsumr[:], rsum[:])
        for c in range(2):
            for et in range(KT4):
                pr = ps.tile([P, 512], dt, tag="s", name=f"pr{c}{et}")
                nc.tensor.matmul(
                    pr[:],
                    lhsT=sel_s[:, et, :],
                    rhs=rsumr[:, c * 512:(c + 1) * 512],
                    start=True,
                    stop=True,
                )
                nc.vector.tensor_tensor(
                    outT[:, et, c * 512:(c + 1) * 512],
                    outT[:, et, c * 512:(c + 1) * 512],
                    pr[:],
                    mult,
                )

        if debug:
            nc.sync.dma_start(dQT.ap(), QT[:])
            nc.sync.dma_start(dKT.ap(), KTt[:])
            nc.sync.dma_start(dVb.ap(), Vb[:])
            nc.sync.dma_start(dsums.ap(), sums[64:64 + HEADS, :])
            nc.sync.dma_start(drsum.ap(), rsum[:])
            nc.sync.dma_start(doutT.ap(), outT[:])

        # ---- output projection ----
        for m in range(8):
            py = ps.tile([P, 512], dt, tag="s", name=f"py{m}")
            for et in range(KT4):
                nc.tensor.matmul(
                    py[:],
                    lhsT=outT[:, et, m * P:(m + 1) * P],
                    rhs=wo_s[:, et, :],
                    start=et == 0,
                    stop=et == KT4 - 1,
                )
            yt = yp.tile([P, 512], dt, tag="y", name=f"yt{m}")
            nc.vector.tensor_tensor(yt[:], py[:], bor_s[:], add)
            nc.sync.dma_start(y_d[m * P:(m + 1) * P, :], yt[:])

    nc.finalize()
    return nc


def _get_program(debug=False):
    key = ("nc", debug)
    if key not in _CACHE:
        _CACHE[key] = _build_program(debug)
    return _CACHE[key]


def _host_inputs(x, Wq, bq, Wk, bk, Wv, bv, Wo, bo):
    f32 = np.float32
    wqT = np.ascontiguousarray(np.asarray(Wq, f32).T).reshape(KT4, P, EMB)
    wkT = np.ascontiguousarray(np.asarray(Wk, f32).T).reshape(KT4, P, EMB)
    wvT = np.ascontiguousarray(np.asarray(Wv, f32).T).reshape(KT4, P, EMB)
    woT = np.ascontiguousarray(np.asarray(Wo, f32).T).reshape(KT4, P, EMB)
    bq2 = np.ascontiguousarray(np.asarray(bq, f32).reshape(KT4, P).T)
    bk2 = np.ascontiguousarray(np.asarray(bk, f32).reshape(KT4, P).T)
    bvr = np.ascontiguousarray(np.tile(np.asarray(bv, f32), (P, 1)))
    bor = np.ascontiguousarray(np.tile(np.asarray(bo, f32), (P, 1)))
    sel = np.zeros((HEADS, KT4, P), f32)
    for et in range(KT4):
        for m in range(P):
            sel[et * 2 + m // D, et, m] = 1.0

    shared = dict(wq=wqT, wk=wkT, wv=wvT, wo=woT, bq2=bq2, bk2=bk2,
                  bvr=bvr, bor=bor, sel=sel)
    x = np.asarray(x, f32)
    in_maps = []
    for c in range(NCORES):
        b, hf = c // 2, c % 2
        xb = x[b]
        # queries first; key order is irrelevant as long as K and V agree
        xr = np.concatenate(
            [xb[hf * HALF:(hf + 1) * HALF], xb[(1 - hf) * HALF:(2 - hf) * HALF]], 0
        )
        xT = np.ascontiguousarray(xr.T).reshape(KT4, P, N)
        in_maps.append(dict(shared, xT=xT))
    return in_maps


def kernel(x, Wq, bq, Wk, bk, Wv, bv, Wo, bo, _trace=False, _trace_cores=None,
           _debug=False):
    from concourse.bass_utils import run_bass_kernel_spmd

    nc = _get_program(_debug)
    in_maps = _host_inputs(x, Wq, bq, Wk, bk, Wv, bv, Wo, bo)
    res = run_bass_kernel_spmd(
        nc, in_maps, list(range(NCORES)), trace=_trace,
        trace_cores=_trace_cores,
    )
    out = np.empty((B, N, EMB), np.float32)
    for c in range(NCORES):
        b, hf = c // 2, c % 2
        out[b, hf * HALF:(hf + 1) * HALF] = res.results[c]["y"]
    if _trace:
        _CACHE["last_results"] = res
    return out



# revision 13
# speedup vs baseline: 1.2325x; 1.2325x over previous
"""Multi-head attention (EMB=512, HEADS=8, x:(4,2048,512)) on 8 Trainium2 cores.

Sharding: zero-collective split — core c handles batch c//2, query rows
(c%2)*1024..(c%2+1)*1024, ALL heads.  K/V projections for the full batch are
computed redundantly on the 2 cores sharing a batch.

v2 vs v1: the kernel is ScalarE-bound (the 16.8M exps per core can only run
on ACT at 1 elem/lane/cycle ~= 139us).  v1 ran projections as serial phases
between attention blocks, leaving ACT idle ~45% of the time.  v2:
  * all matmul operands bf16 (host-converted): halves DMA + SBUF, enables FWL
  * DMA ordered critical-first so the first exp fires at ~6us
  * every projection matmul is interleaved into the attention g-step stream
    as "filler" work with deadlines, so ACT streams exp back-to-back
  * PV lags the exp stream by PV_LAG steps (deep pt pool) so the V-projection
    bulge at the start doesn't starve ACT

Device dataflow per core (all SBUF-resident):
  xtc[cc]                  [128, 4, 512] bf16  token chunks, queries first
  Q^T = WqT.T @ x^T (+bq)  [128, 4, 1024] bf16 feature-major
  K^T = WkT.T @ x^T (+bk)  [128, 4, 2048] bf16 feature-major
  V~  = x^T.T @ WvT (+bv)  [128, 16, 8, 72] bf16 token-major, per-head 1s col
  S^T = K^T_h.T @ Q^T_h    per (head, 2-key-tile, 512-query chunk) -> PSUM
  P^T = exp(S^T/sqrt(512)) ScalarE fused drain, bf16
  outT~ = V~_h.T @ P^T     [72, 512] PSUM accum over 16 key tiles
  R = sel.T @ recip(sums)  partition-broadcast of 1/denominator via PE
  outT = outT~ * R         [128, 4, 1024] bf16
  y = outT.T @ WoT (+bo)   [1024, 512] fp32, DMA out
"""

import sys
import os

for _p in ("/opt/trn_rl_repo", "/root/.axon_site/_ro/trn_rl_repo"):
    if os.path.isdir(_p) and _p not in sys.path:
        sys.path.append(_p)

import numpy as np

EMB = 512
HEADS = 8
D = 64  # head dim
B = 4
N = 2048  # keys / tokens per batch
HALF = 1024  # queries per core
P = 128
NCORES = 8
KT4 = EMB // P  # 4 contraction tiles
SCALE = float(1.0 / np.sqrt(np.float32(EMB)))
PV_LAG = 3  # attention steps PV trails the exp stream by

_CACHE = {}


def _build_program(debug=False):
    from concourse import bacc
    import concourse.mybir as mybir
    import concourse.tile as tile
    from contextlib import ExitStack

    f32 = mybir.dt.float32
    f32r = mybir.dt.float32r
    bf16 = mybir.dt.bfloat16
    nc = bacc.Bacc("TRN2", target_bir_lowering=False)

    # ---- DRAM tensors (order matters only via emission order of dma_start) ----
    xc_d = nc.dram_tensor("xc", [4, P, KT4, 512], bf16, kind="ExternalInput")
    wq_d = nc.dram_tensor("wq", [P, KT4, EMB], bf16, kind="ExternalInput")
    wk_d = nc.dram_tensor("wk", [P, KT4, EMB], bf16, kind="ExternalInput")
    wv_d = nc.dram_tensor("wv", [P, KT4, EMB], bf16, kind="ExternalInput")
    wo_d = nc.dram_tensor("wo", [P, KT4, EMB], bf16, kind="ExternalInput")
    bq_d = nc.dram_tensor("bq2", [P, KT4], f32, kind="ExternalInput")
    bk_d = nc.dram_tensor("bk2", [P, KT4], f32, kind="ExternalInput")
    bvr_d = nc.dram_tensor("bvr", [P, EMB], f32, kind="ExternalInput")
    bor_d = nc.dram_tensor("bor", [P, EMB], f32, kind="ExternalInput")
    sel_d = nc.dram_tensor("sel", [HEADS, KT4, P], f32r, kind="ExternalInput")
    y_d = nc.dram_tensor("y", [HALF, EMB], f32, kind="ExternalOutput")
    if debug:
        dQT = nc.dram_tensor("dQT", [P, KT4, HALF], bf16, kind="ExternalOutput")
        dKT = nc.dram_tensor("dKT", [P, KT4, N], bf16, kind="ExternalOutput")
        dVb = nc.dram_tensor("dVb", [P, 16, HEADS, D + 8], bf16,
                             kind="ExternalOutput")
        dsums = nc.dram_tensor("dsums", [HEADS, HALF], f32, kind="ExternalOutput")
        drsum = nc.dram_tensor("drsum", [HEADS, HALF], f32, kind="ExternalOutput")
        dsums0_t = nc.dram_tensor("dsums0", [HEADS, 512], f32,
                                  kind="ExternalOutput")
        drsum_t = nc.dram_tensor("drsum0", [HEADS, 512], f32,
                                 kind="ExternalOutput")
        drsumr_t = nc.dram_tensor("drsumr0", [HEADS, 512], f32r,
                                  kind="ExternalOutput")
        dsums_t = nc.dram_tensor("dsums_t", [HEADS, 512], f32,
                                 kind="ExternalOutput")
        doutT = nc.dram_tensor("doutT", [P, KT4, HALF], bf16,
                               kind="ExternalOutput")

    Exp = mybir.ActivationFunctionType.Exp
    mult = mybir.AluOpType.mult
    add = mybir.AluOpType.add

    with tile.TileContext(nc) as tc, ExitStack() as ctx:
        xp = ctx.enter_context(tc.tile_pool(name="xp", bufs=1))
        wp = ctx.enter_context(tc.tile_pool(name="wp", bufs=1))
        pers = ctx.enter_context(tc.tile_pool(name="pers", bufs=1))
        ptp = ctx.enter_context(tc.tile_pool(name="ptp", bufs=12))
        yp = ctx.enter_context(tc.tile_pool(name="yp", bufs=2))
        # PSUM: tag "s" 3 x [128,1024] (6 banks) + tag "pv" 2 x 1 bank
        ps = ctx.enter_context(tc.tile_pool(name="ps", bufs=3, space="PSUM"))

        # ---- input DMAs, critical-first ----
        wk_s = wp.tile([P, KT4, EMB], bf16, name="wks", tag="wks")
        nc.sync.dma_start(wk_s[:], wk_d[:])
        wq_s = wp.tile([P, KT4, EMB], bf16, name="wqs", tag="wqs")
        nc.sync.dma_start(wq_s[:], wq_d[:])
        xtc = []
        for cc in range(4):
            t = xp.tile([P, KT4, 512], bf16, name=f"xc{cc}", tag=f"xc{cc}")
            xtc.append(t)
        nc.sync.dma_start(xtc[0][:], xc_d[0])
        bk_s = pers.tile([P, KT4], f32, name="bks")
        nc.sync.dma_start(bk_s[:], bk_d[:])
        bq_s = pers.tile([P, KT4], f32, name="bqs")
        nc.sync.dma_start(bq_s[:], bq_d[:])
        nc.sync.dma_start(xtc[1][:], xc_d[1])
        wv_s = wp.tile([P, KT4, EMB], bf16, name="wvs", tag="wvs")
        nc.sync.dma_start(wv_s[:], wv_d[:])
        bvr_s = pers.tile([P, HEADS, D], f32, name="bvrs")
        nc.sync.dma_start(bvr_s[:], bvr_d.ap().rearrange("p (h d) -> p h d", d=D))
        nc.sync.dma_start(xtc[2][:], xc_d[2])
        nc.sync.dma_start(xtc[3][:], xc_d[3])
        wo_s = wp.tile([P, KT4, EMB], bf16, name="wos", tag="wos")
        nc.sync.dma_start(wo_s[:], wo_d[:])
        bor_s = pers.tile([P, EMB], f32, name="bors")
        nc.sync.dma_start(bor_s[:], bor_d[:])
        sel_s = pers.tile([HEADS, KT4, P], f32r, name="sels")
        nc.sync.dma_start(sel_s[:], sel_d[:])

        # ---- persistent intermediates ----
        QT = pers.tile([P, KT4, HALF], bf16, name="QT")
        KTt = pers.tile([P, KT4, N], bf16, name="KTt")
        Vb = pers.tile([P, 16, HEADS, D + 8], bf16, name="Vb")
        outT = pers.tile([P, KT4, HALF], bf16, name="outT")
        sums = pers.tile([64 + HEADS, HALF], f32, name="sums")
        sums0 = pers.tile([HEADS, HALF], f32, name="sums0")
        rsum = pers.tile([HEADS, HALF], f32, name="rsum")
        rsumr = pers.tile([HEADS, HALF], f32r, name="rsumr")

        # per-head one-hot indicator columns: PV lands head h's softmax
        # denominator on PSUM partition 64+h
        nc.vector.memset(Vb[:, :, :, D:D + 8], 0.0)
        for h in range(HEADS):
            nc.vector.memset(Vb[:, :, h, D + h], 1.0)
        nc.vector.memset(sums[64:64 + HEADS, :], 0.0)

        # ---- granule emitters (each uses one "s" PSUM slot) ----
        def emit_k_half(jt, cc):
            pk = ps.tile([P, 1024], f32, tag="s", name=f"pk{jt}{cc}")
            for kt in range(KT4):
                nc.tensor.matmul(
                    pk[:, 0:512],
                    lhsT=wk_s[:, kt, jt * P:(jt + 1) * P],
                    rhs=xtc[cc][:, kt, :],
                    start=kt == 0,
                    stop=kt == KT4 - 1,
                )
            nc.vector.tensor_scalar_add(
                KTt[:, jt, cc * 512:(cc + 1) * 512], pk[:, 0:512],
                bk_s[:, jt:jt + 1],
            )

        def emit_k_pair(jt, ccp):
            pk = ps.tile([P, 1024], f32, tag="s", name=f"pkp{jt}{ccp}")
            for i, cc in enumerate((2 * ccp, 2 * ccp + 1)):
                for kt in range(KT4):
                    nc.tensor.matmul(
                        pk[:, i * 512:(i + 1) * 512],
                        lhsT=wk_s[:, kt, jt * P:(jt + 1) * P],
                        rhs=xtc[cc][:, kt, :],
                        start=kt == 0,
                        stop=kt == KT4 - 1,
                    )
                nc.vector.tensor_scalar_add(
                    KTt[:, jt, cc * 512:(cc + 1) * 512], pk[:, i * 512:(i + 1) * 512],
                    bk_s[:, jt:jt + 1],
                )

        def emit_q_half(jt, c):
            pq = ps.tile([P, 1024], f32, tag="s", name=f"pq{jt}{c}")
            for kt in range(KT4):
                nc.tensor.matmul(
                    pq[:, 0:512],
                    lhsT=wq_s[:, kt, jt * P:(jt + 1) * P],
                    rhs=xtc[c][:, kt, :],
                    start=kt == 0,
                    stop=kt == KT4 - 1,
                )
            nc.vector.tensor_scalar_add(
                QT[:, jt, c * 512:(c + 1) * 512], pq[:, 0:512],
                bq_s[:, jt:jt + 1],
            )

        def emit_v_pair(vg):
            # V projection for key tiles t = 2*vg, 2*vg+1
            pv = ps.tile([P, 1024], f32, tag="s", name=f"pvv{vg}")
            for i, t in enumerate((2 * vg, 2 * vg + 1)):
                cc, lo = t // 4, (t % 4) * P
                for kt in range(KT4):
                    nc.tensor.matmul(
                        pv[:, i * 512:(i + 1) * 512],
                        lhsT=xtc[cc][:, kt, lo:lo + P],
                        rhs=wv_s[:, kt, :],
                        start=kt == 0,
                        stop=kt == KT4 - 1,
                    )
                nc.vector.tensor_tensor(
                    Vb[:, t, :, 0:D],
                    pv[:, i * 512:(i + 1) * 512].rearrange("p (h d) -> p h d", d=D),
                    bvr_s[:],
                    add,
                )

        def emit_norm_scalar(c):
            cs = slice(c * 512, (c + 1) * 512)
            if debug and c == 0:
                nc.sync.dma_start(dsums_t.ap(), sums[64:64 + HEADS, cs])
            nc.vector.tensor_copy(sums0[:, cs], sums[64:64 + HEADS, cs])
            nc.vector.reciprocal_approx_fast(rsum[:, cs], sums0[:, cs])
            nc.vector.tensor_copy(rsumr[:, cs], rsum[:, cs])
            if debug and c == 0:
                nc.sync.dma_start(dsums0_t.ap(), sums0[:, cs])
                nc.sync.dma_start(drsum_t.ap(), rsum[:, cs])
                nc.sync.dma_start(drsumr_t.ap(), rsumr[:, cs])

        def emit_norm_pair(c, ep):
            cs = slice(c * 512, (c + 1) * 512)
            pr = ps.tile([P, 1024], f32, tag="s", name=f"pr{c}{ep}")
            for i, et in enumerate((2 * ep, 2 * ep + 1)):
                nc.tensor.matmul(
                    pr[:, i * 512:(i + 1) * 512],
                    lhsT=sel_s[:, et, :],
                    rhs=rsumr[:, cs],
                    start=True,
                    stop=True,
                )
                nc.vector.tensor_tensor(
                    outT[:, et, cs],
                    outT[:, et, cs],
                    pr[:, i * 512:(i + 1) * 512],
                    mult,
                )

        def emit_oproj_pair(c, q):
            py = ps.tile([P, 1024], f32, tag="s", name=f"py{c}{q}")
            for i in range(2):
                m = c * 4 + q * 2 + i
                for et in range(KT4):
                    nc.tensor.matmul(
                        py[:, i * 512:(i + 1) * 512],
                        lhsT=outT[:, et, m * P:(m + 1) * P],
                        rhs=wo_s[:, et, :],
                        start=et == 0,
                        stop=et == KT4 - 1,
                    )
                yt = yp.tile([P, 512], f32, tag="y", name=f"yt{c}{q}{i}")
                nc.vector.tensor_tensor(yt[:], py[:, i * 512:(i + 1) * 512],
                                        bor_s[:], add)
                nc.sync.dma_start(y_d[m * P:(m + 1) * P, :], yt[:])

        # ---- attention pieces ----
        pv_acc = {}  # (hp, c) -> (pvA, pvB)

        def emit_s_exp(hp, c, g):
            cs = slice(c * 512, (c + 1) * 512)
            sA = ps.tile([P, 1024], f32, tag="s", name=f"sA{hp}{c}{g}")
            for tt in range(2):
                t = 2 * g + tt
                nc.tensor.matmul(
                    sA[:, tt * 512:(tt + 1) * 512],
                    lhsT=KTt[0:D, hp, t * P:(t + 1) * P],
                    rhs=QT[0:D, hp, cs],
                    start=True,
                    stop=True,
                )
            sB = ps.tile([P, 1024], f32, tag="s", name=f"sB{hp}{c}{g}")
            for tt in range(2):
                t = 2 * g + tt
                nc.tensor.matmul(
                    sB[:, tt * 512:(tt + 1) * 512],
                    lhsT=KTt[D:P, hp, t * P:(t + 1) * P],
                    rhs=QT[D:P, hp, cs],
                    start=True,
                    stop=True,
                )
            ptA = ptp.tile([P, 1024], bf16, tag="pt", name=f"ptA{hp}{c}{g}")
            nc.scalar.activation(ptA[:], sA[:], Exp, scale=SCALE)
            ptB = ptp.tile([P, 1024], bf16, tag="pt", name=f"ptB{hp}{c}{g}")
            nc.scalar.activation(ptB[:], sB[:], Exp, scale=SCALE)
            return ptA, ptB

        def emit_pv(hp, c, g, ptA, ptB):
            hA, hB = 2 * hp, 2 * hp + 1
            if g == 0:
                pv_acc[(hp, c)] = (
                    ps.tile([D + 8, 512], f32, tag="pv", bufs=2,
                            name=f"pvA{hp}{c}"),
                    ps.tile([D + 8, 512], f32, tag="pv", bufs=2,
                            name=f"pvB{hp}{c}"),
                )
            pvA, pvB = pv_acc[(hp, c)]
            for tt in range(2):
                t = 2 * g + tt
                nc.tensor.matmul(
                    pvA[:],
                    lhsT=Vb[:, t, hA, :],
                    rhs=ptA[:, tt * 512:(tt + 1) * 512],
                    start=t == 0,
                    stop=t == 15,
                )
                nc.tensor.matmul(
                    pvB[:],
                    lhsT=Vb[:, t, hB, :],
                    rhs=ptB[:, tt * 512:(tt + 1) * 512],
                    start=t == 0,
                    stop=t == 15,
                )
            if g == 7:
                cs = slice(c * 512, (c + 1) * 512)
                for pv_, h in ((pvA, hA), (pvB, hB)):
                    po = (h % 2) * D
                    nc.vector.tensor_copy(outT[po:po + D, h // 2, cs], pv_[0:D, :])
                    nc.vector.tensor_tensor(
                        sums[64:64 + HEADS, cs],
                        sums[64:64 + HEADS, cs],
                        pv_[D:D + 8, :],
                        add,
                    )

        # ---- filler queue: (deadline_step, earliest_step, closure) ----
        # deadline: must be emitted by this step (consumer correctness).
        # earliest: must NOT be emitted before this step — for the normalize/
        # output chain it guards in-place accumulation order (sums/outT);
        # for DMA-fed granules it avoids head-of-line PE stalls on DMA.
        fillers = []

        def F(dl, early, fn, *a):
            fillers.append((dl, early, lambda: fn(*a)))

        F(2, 1, emit_k_half, 0, 1)       # keys 512-1023 for g=2,3
        F(4, 3, emit_k_pair, 0, 1)       # keys 1024-2047 for g>=4
        for vg in range(8):              # V for PV(hp0,c0,g=vg) at step vg+PV_LAG
            F(max(0, vg + PV_LAG - 1), min(2 + vg // 2, 6), emit_v_pair, vg)
        F(7, 2, emit_q_half, 0, 1)       # queries 512-1023 for step 8
        for jt in range(1, KT4):
            base = 16 * jt
            F(base - 4, 8, emit_k_pair, jt, 0)
            F(base - 3, 8, emit_k_pair, jt, 1)
            F(base - 2, 8, emit_q_half, jt, 0)
            F(base + 6, 8, emit_q_half, jt, 1)
        F(59, 59, emit_norm_scalar, 0)
        F(60, 60, emit_norm_pair, 0, 0)
        F(60, 60, emit_norm_pair, 0, 1)
        F(61, 61, emit_oproj_pair, 0, 0)
        F(62, 62, emit_oproj_pair, 0, 1)
        # tail (deadline > 63): emitted after the main loop
        F(64, 64, emit_norm_scalar, 1)
        F(64, 64, emit_norm_pair, 1, 0)
        F(64, 64, emit_norm_pair, 1, 1)
        F(65, 65, emit_oproj_pair, 1, 0)
        F(65, 65, emit_oproj_pair, 1, 1)
        fillers.sort(key=lambda x: x[0])

        # ---- warmup: first K/Q granules so attention can start ASAP ----
        emit_k_half(0, 0)
        emit_q_half(0, 0)

        # ---- main interleaved loop ----
        n_early = sum(1 for dl, _, _ in fillers if dl < 64)
        emitted = 0
        pv_pending = []
        step = 0
        for hp in range(KT4):
            for c in range(2):
                for g in range(8):
                    # forced fillers (deadline), then smooth drain (respects
                    # each filler's earliest bound)
                    while fillers and fillers[0][0] <= step:
                        fillers.pop(0)[2]()
                        emitted += 1
                    target = (n_early * (step + 1) + 63) // 64
                    while (fillers and emitted < target and fillers[0][0] < 64
                           and fillers[0][1] <= step):
                        fillers.pop(0)[2]()
                        emitted += 1
                    ptA, ptB = emit_s_exp(hp, c, g)
                    pv_pending.append((hp, c, g, ptA, ptB))
                    if len(pv_pending) > PV_LAG:
                        emit_pv(*pv_pending.pop(0))
                    step += 1
        # ---- tail: flush PV, then remaining fillers (norm + oproj c1) ----
        while pv_pending:
            emit_pv(*pv_pending.pop(0))
        if debug:
            nc.sync.dma_start(dsums.ap(), sums[64:64 + HEADS, :])
        while fillers:
            fillers.pop(0)[2]()
        if debug:
            nc.sync.dma_start(dQT.ap(), QT[:])
            nc.sync.dma_start(dKT.ap(), KTt[:])
            nc.sync.dma_start(dVb.ap(), Vb[:])
            nc.sync.dma_start(drsum.ap(), rsum[:])
            nc.sync.dma_start(doutT.ap(), outT[:])

    nc.finalize()
    return nc


def _get_program(debug=False):
    key = ("nc", debug)
    if key not in _CACHE:
        _CACHE[key] = _build_program(debug)
    return _CACHE[key]


def _host_inputs(x, Wq, bq, Wk, bk, Wv, bv, Wo, bo):
    import ml_dtypes

    f32 = np.float32
    bf16 = ml_dtypes.bfloat16

    def prep_w(W):
        # [in_feat, out_feat] -> [P, KT4, EMB] bf16 (in_feat = kt*128 + p)
        t = np.asarray(W, f32).T.reshape(KT4, P, EMB).transpose(1, 0, 2)
        return np.ascontiguousarray(t).astype(bf16)

    wq = prep_w(Wq)
    wk = prep_w(Wk)
    wv = prep_w(Wv)
    wo = prep_w(Wo)
    bq2 = np.ascontiguousarray(np.asarray(bq, f32).reshape(KT4, P).T)
    bk2 = np.ascontiguousarray(np.asarray(bk, f32).reshape(KT4, P).T)
    bvr = np.ascontiguousarray(np.tile(np.asarray(bv, f32), (P, 1)))
    bor = np.ascontiguousarray(np.tile(np.asarray(bo, f32), (P, 1)))
    sel = np.zeros((HEADS, KT4, P), f32)
    for et in range(KT4):
        for m in range(P):
            sel[et * 2 + m // D, et, m] = 1.0

    shared = dict(wq=wq, wk=wk, wv=wv, wo=wo, bq2=bq2, bk2=bk2,
                  bvr=bvr, bor=bor, sel=sel)
    x = np.asarray(x, f32)
    in_maps = []
    for core in range(NCORES):
        b, hf = core // 2, core % 2
        xb = x[b]
        # queries first; key order is irrelevant as long as K and V agree
        xr = np.concatenate(
            [xb[hf * HALF:(hf + 1) * HALF], xb[(1 - hf) * HALF:(2 - hf) * HALF]], 0
        )
        # [tokens, feat] -> [4cc, P, KT4, 512]: feat = kt*128+p, token = cc*512+j
        xT = xr.T.reshape(KT4, P, N).transpose(1, 0, 2)  # [P, KT4, N]
        xc = xT.reshape(P, KT4, 4, 512).transpose(2, 0, 1, 3)
        in_maps.append(dict(shared, xc=np.ascontiguousarray(xc).astype(bf16)))
    return in_maps


def kernel(x, Wq, bq, Wk, bk, Wv, bv, Wo, bo, _trace=False, _trace_cores=None,
           _debug=False):
    from concourse.bass_utils import run_bass_kernel_spmd

    nc = _get_program(_debug)
    in_maps = _host_inputs(x, Wq, bq, Wk, bk, Wv, bv, Wo, bo)
    res = run_bass_kernel_spmd(
        nc, in_maps, list(range(NCORES)), trace=_trace,
        trace_cores=_trace_cores,
    )
    out = np.empty((B, N, EMB), np.float32)
    for core in range(NCORES):
        b, hf = core // 2, core % 2
        out[b, hf * HALF:(hf + 1) * HALF] = res.results[core]["y"]
    if _trace:
        _CACHE["last_results"] = res
    return out


# revision 16
# speedup vs baseline: 1.3143x; 1.0664x over previous
"""Multi-head attention (EMB=512, HEADS=8, x:(4,2048,512)) on 8 Trainium2 cores.

Sharding: zero-collective split — core c handles batch c//2, query rows
(c%2)*1024..(c%2+1)*1024, ALL heads.  K/V projections for the full batch are
computed redundantly on the 2 cores sharing a batch.

v2 vs v1: the kernel is ScalarE-bound (the 16.8M exps per core can only run
on ACT at 1 elem/lane/cycle ~= 139us).  v1 ran projections as serial phases
between attention blocks, leaving ACT idle ~45% of the time.  v2:
  * all matmul operands bf16 (host-converted): halves DMA + SBUF, enables FWL
  * DMA ordered critical-first so the first exp fires at ~6us
  * every projection matmul is interleaved into the attention g-step stream
    as "filler" work with deadlines, so ACT streams exp back-to-back
  * PV lags the exp stream by PV_LAG steps (deep pt pool) so the V-projection
    bulge at the start doesn't starve ACT

Device dataflow per core (all SBUF-resident):
  xtc[cc]                  [128, 4, 512] bf16  token chunks, queries first
  Q^T = WqT.T @ x^T (+bq)  [128, 4, 1024] bf16 feature-major
  K^T = WkT.T @ x^T (+bk)  [128, 4, 2048] bf16 feature-major
  V~  = x^T.T @ WvT (+bv)  [128, 16, 8, 72] bf16 token-major, per-head 1s col
  S^T = K^T_h.T @ Q^T_h    per (head, 2-key-tile, 512-query chunk) -> PSUM
  P^T = exp(S^T/sqrt(512)) ScalarE fused drain, bf16
  outT~ = V~_h.T @ P^T     [72, 512] PSUM accum over 16 key tiles
  R = sel.T @ recip(sums)  partition-broadcast of 1/denominator via PE
  outT = outT~ * R         [128, 4, 1024] bf16
  y = outT.T @ WoT (+bo)   [1024, 512] fp32, DMA out
"""

import sys
import os

for _p in ("/opt/trn_rl_repo", "/root/.axon_site/_ro/trn_rl_repo"):
    if os.path.isdir(_p) and _p not in sys.path:
        sys.path.append(_p)

import numpy as np

EMB = 512
HEADS = 8
D = 64  # head dim
B = 4
N = 2048  # keys / tokens per batch
HALF = 1024  # queries per core
P = 128
NCORES = 8
KT4 = EMB // P  # 4 contraction tiles
SCALE = float(1.0 / np.sqrt(np.float32(EMB)))
PV_LAG = 3  # attention steps PV trails the exp stream by

_CACHE = {}


def _build_program(debug=False):
    from concourse import bacc
    import concourse.mybir as mybir
    import concourse.tile as tile
    from contextlib import ExitStack

    f32 = mybir.dt.float32
    f32r = mybir.dt.float32r
    bf16 = mybir.dt.bfloat16
    nc = bacc.Bacc("TRN2", target_bir_lowering=False)

    # ---- DRAM tensors (order matters only via emission order of dma_start) ----
    xc_d = nc.dram_tensor("xc", [4, P, KT4, 512], bf16, kind="ExternalInput")
    wq_d = nc.dram_tensor("wq", [P, KT4, EMB], bf16, kind="ExternalInput")
    wk_d = nc.dram_tensor("wk", [P, KT4, EMB], bf16, kind="ExternalInput")
    wv_d = nc.dram_tensor("wv", [P, KT4, EMB], bf16, kind="ExternalInput")
    wo_d = nc.dram_tensor("wo", [P, KT4, EMB], bf16, kind="ExternalInput")
    bq_d = nc.dram_tensor("bq2", [P, KT4], f32, kind="ExternalInput")
    bk_d = nc.dram_tensor("bk2", [P, KT4], f32, kind="ExternalInput")
    bvr_d = nc.dram_tensor("bvr", [P, EMB], f32, kind="ExternalInput")
    bor_d = nc.dram_tensor("bor", [P, EMB], f32, kind="ExternalInput")
    sel_d = nc.dram_tensor("sel", [HEADS, KT4, P], f32r, kind="ExternalInput")
    y_d = nc.dram_tensor("y", [HALF, EMB], f32, kind="ExternalOutput")
    if debug:
        dQT = nc.dram_tensor("dQT", [P, KT4, HALF], bf16, kind="ExternalOutput")
        dKT = nc.dram_tensor("dKT", [P, KT4, N], bf16, kind="ExternalOutput")
        dVb = nc.dram_tensor("dVb", [P, 16, HEADS, D + 8], bf16,
                             kind="ExternalOutput")
        dsums = nc.dram_tensor("dsums", [HEADS, HALF], f32, kind="ExternalOutput")
        drsum = nc.dram_tensor("drsum", [HEADS, HALF], f32, kind="ExternalOutput")
        dsums0_t = nc.dram_tensor("dsums0", [HEADS, 512], f32,
                                  kind="ExternalOutput")
        drsum_t = nc.dram_tensor("drsum0", [HEADS, 512], f32,
                                 kind="ExternalOutput")
        drsumr_t = nc.dram_tensor("drsumr0", [HEADS, 512], f32r,
                                  kind="ExternalOutput")
        dsums_t = nc.dram_tensor("dsums_t", [HEADS, 512], f32,
                                 kind="ExternalOutput")
        doutT = nc.dram_tensor("doutT", [P, KT4, HALF], bf16,
                               kind="ExternalOutput")

    Exp = mybir.ActivationFunctionType.Exp
    mult = mybir.AluOpType.mult
    add = mybir.AluOpType.add

    with tile.TileContext(nc) as tc, ExitStack() as ctx:
        xp = ctx.enter_context(tc.tile_pool(name="xp", bufs=1))
        wp = ctx.enter_context(tc.tile_pool(name="wp", bufs=1))
        pers = ctx.enter_context(tc.tile_pool(name="pers", bufs=1))
        ptp = ctx.enter_context(tc.tile_pool(name="ptp", bufs=12))
        yp = ctx.enter_context(tc.tile_pool(name="yp", bufs=2))
        # PSUM: tag "s" 3 x [128,1024] (6 banks) + tag "pv" 2 x 1 bank
        ps = ctx.enter_context(tc.tile_pool(name="ps", bufs=3, space="PSUM"))

        # ---- input DMAs, critical-first ----
        wk_s = wp.tile([P, KT4, EMB], bf16, name="wks", tag="wks")
        nc.sync.dma_start(wk_s[:], wk_d[:])
        wq_s = wp.tile([P, KT4, EMB], bf16, name="wqs", tag="wqs")
        nc.sync.dma_start(wq_s[:], wq_d[:])
        xtc = []
        for cc in range(4):
            t = xp.tile([P, KT4, 512], bf16, name=f"xc{cc}", tag=f"xc{cc}")
            xtc.append(t)
        nc.sync.dma_start(xtc[0][:], xc_d[0])
        bk_s = pers.tile([P, KT4], f32, name="bks")
        nc.sync.dma_start(bk_s[:], bk_d[:])
        bq_s = pers.tile([P, KT4], f32, name="bqs")
        nc.sync.dma_start(bq_s[:], bq_d[:])
        nc.sync.dma_start(xtc[1][:], xc_d[1])
        wv_s = wp.tile([P, KT4, EMB], bf16, name="wvs", tag="wvs")
        nc.sync.dma_start(wv_s[:], wv_d[:])
        bvr_s = pers.tile([P, HEADS, D], f32, name="bvrs")
        nc.sync.dma_start(bvr_s[:], bvr_d.ap().rearrange("p (h d) -> p h d", d=D))
        nc.sync.dma_start(xtc[2][:], xc_d[2])
        nc.sync.dma_start(xtc[3][:], xc_d[3])
        wo_s = wp.tile([P, KT4, EMB], bf16, name="wos", tag="wos")
        nc.sync.dma_start(wo_s[:], wo_d[:])
        bor_s = pers.tile([P, EMB], f32, name="bors")
        nc.sync.dma_start(bor_s[:], bor_d[:])
        sel_s = pers.tile([HEADS, KT4, P], f32r, name="sels")
        nc.sync.dma_start(sel_s[:], sel_d[:])

        # ---- persistent intermediates ----
        QT = pers.tile([P, KT4, HALF], bf16, name="QT")
        KTt = pers.tile([P, KT4, N], bf16, name="KTt")
        Vb = pers.tile([P, 16, HEADS, D + 8], bf16, name="Vb")
        outT = pers.tile([P, KT4, HALF], bf16, name="outT")
        sums = pers.tile([64 + HEADS, HALF], f32, name="sums")
        sums0 = pers.tile([HEADS, HALF], f32, name="sums0")
        rsum = pers.tile([HEADS, HALF], f32, name="rsum")
        rsumr = pers.tile([HEADS, HALF], f32r, name="rsumr")

        # per-head one-hot indicator columns: PV lands head h's softmax
        # denominator on PSUM partition 64+h
        nc.vector.memset(Vb[:, :, :, D:D + 8], 0.0)
        for h in range(HEADS):
            nc.vector.memset(Vb[:, :, h, D + h], 1.0)
        nc.vector.memset(sums[64:64 + HEADS, :], 0.0)

        # ---- granule emitters (each uses one "s" PSUM slot) ----
        def emit_k_half(jt, cc):
            pk = ps.tile([P, 1024], f32, tag="s", name=f"pk{jt}{cc}")
            for kt in range(KT4):
                nc.tensor.matmul(
                    pk[:, 0:512],
                    lhsT=wk_s[:, kt, jt * P:(jt + 1) * P],
                    rhs=xtc[cc][:, kt, :],
                    start=kt == 0,
                    stop=kt == KT4 - 1,
                )
            nc.vector.tensor_scalar_add(
                KTt[:, jt, cc * 512:(cc + 1) * 512], pk[:, 0:512],
                bk_s[:, jt:jt + 1],
            )

        def emit_k_pair(jt, ccp):
            pk = ps.tile([P, 1024], f32, tag="s", name=f"pkp{jt}{ccp}")
            for i, cc in enumerate((2 * ccp, 2 * ccp + 1)):
                for kt in range(KT4):
                    nc.tensor.matmul(
                        pk[:, i * 512:(i + 1) * 512],
                        lhsT=wk_s[:, kt, jt * P:(jt + 1) * P],
                        rhs=xtc[cc][:, kt, :],
                        start=kt == 0,
                        stop=kt == KT4 - 1,
                    )
                nc.vector.tensor_scalar_add(
                    KTt[:, jt, cc * 512:(cc + 1) * 512], pk[:, i * 512:(i + 1) * 512],
                    bk_s[:, jt:jt + 1],
                )

        def emit_q_half(jt, c):
            pq = ps.tile([P, 1024], f32, tag="s", name=f"pq{jt}{c}")
            for kt in range(KT4):
                nc.tensor.matmul(
                    pq[:, 0:512],
                    lhsT=wq_s[:, kt, jt * P:(jt + 1) * P],
                    rhs=xtc[c][:, kt, :],
                    start=kt == 0,
                    stop=kt == KT4 - 1,
                )
            nc.vector.tensor_scalar_add(
                QT[:, jt, c * 512:(c + 1) * 512], pq[:, 0:512],
                bq_s[:, jt:jt + 1],
            )

        def emit_v_pair(vg):
            # V projection for key tiles t = 2*vg, 2*vg+1
            pv = ps.tile([P, 1024], f32, tag="s", name=f"pvv{vg}")
            for i, t in enumerate((2 * vg, 2 * vg + 1)):
                cc, lo = t // 4, (t % 4) * P
                for kt in range(KT4):
                    nc.tensor.matmul(
                        pv[:, i * 512:(i + 1) * 512],
                        lhsT=xtc[cc][:, kt, lo:lo + P],
                        rhs=wv_s[:, kt, :],
                        start=kt == 0,
                        stop=kt == KT4 - 1,
                    )
                nc.vector.tensor_tensor(
                    Vb[:, t, :, 0:D],
                    pv[:, i * 512:(i + 1) * 512].rearrange("p (h d) -> p h d", d=D),
                    bvr_s[:],
                    add,
                )

        def emit_norm_scalar(c):
            cs = slice(c * 512, (c + 1) * 512)
            if debug and c == 0:
                nc.sync.dma_start(dsums_t.ap(), sums[64:64 + HEADS, cs])
            nc.vector.tensor_copy(sums0[:, cs], sums[64:64 + HEADS, cs])
            nc.vector.reciprocal_approx_fast(rsum[:, cs], sums0[:, cs])
            nc.vector.tensor_copy(rsumr[:, cs], rsum[:, cs])
            if debug and c == 0:
                nc.sync.dma_start(dsums0_t.ap(), sums0[:, cs])
                nc.sync.dma_start(drsum_t.ap(), rsum[:, cs])
                nc.sync.dma_start(drsumr_t.ap(), rsumr[:, cs])

        def emit_norm_pair(c, ep):
            cs = slice(c * 512, (c + 1) * 512)
            pr = ps.tile([P, 1024], f32, tag="s", name=f"pr{c}{ep}")
            for i, et in enumerate((2 * ep, 2 * ep + 1)):
                nc.tensor.matmul(
                    pr[:, i * 512:(i + 1) * 512],
                    lhsT=sel_s[:, et, :],
                    rhs=rsumr[:, cs],
                    start=True,
                    stop=True,
                )
                nc.vector.tensor_tensor(
                    outT[:, et, cs],
                    outT[:, et, cs],
                    pr[:, i * 512:(i + 1) * 512],
                    mult,
                )

        def emit_oproj_pair(c, q):
            py = ps.tile([P, 1024], f32, tag="s", name=f"py{c}{q}")
            for i in range(2):
                m = c * 4 + q * 2 + i
                for et in range(KT4):
                    nc.tensor.matmul(
                        py[:, i * 512:(i + 1) * 512],
                        lhsT=outT[:, et, m * P:(m + 1) * P],
                        rhs=wo_s[:, et, :],
                        start=et == 0,
                        stop=et == KT4 - 1,
                    )
                yt = yp.tile([P, 512], f32, tag="y", name=f"yt{c}{q}{i}")
                nc.vector.tensor_tensor(yt[:], py[:, i * 512:(i + 1) * 512],
                                        bor_s[:], add)
                nc.sync.dma_start(y_d[m * P:(m + 1) * P, :], yt[:])

        # ---- attention pieces ----
        pv_acc = {}  # (hp, c) -> (pvA, pvB)

        def emit_s_exp(hp, c, g):
            # Each PSUM slot holds [head A | head B] scores for ONE key tile,
            # so the two matmuls hit disjoint PE row groups back-to-back and
            # run concurrently (row tiling), and exp can fire after ~1 MM time.
            cs = slice(c * 512, (c + 1) * 512)
            pts = []
            for tt in range(2):
                t = 2 * g + tt
                s_ = ps.tile([P, 1024], f32, tag="s", name=f"s{hp}{c}{g}{tt}")
                nc.tensor.matmul(
                    s_[:, 0:512],
                    lhsT=KTt[0:D, hp, t * P:(t + 1) * P],
                    rhs=QT[0:D, hp, cs],
                    start=True,
                    stop=True,
                )
                nc.tensor.matmul(
                    s_[:, 512:1024],
                    lhsT=KTt[D:P, hp, t * P:(t + 1) * P],
                    rhs=QT[D:P, hp, cs],
                    start=True,
                    stop=True,
                )
                pt = ptp.tile([P, 1024], bf16, tag="pt", name=f"pt{hp}{c}{g}{tt}")
                nc.scalar.activation(pt[:], s_[:], Exp, scale=SCALE)
                pts.append(pt)
            return pts

        def emit_pv(hp, c, g, ptA, ptB):
            # ptA/ptB = the two key-tile P^T tiles of step g, each laid out
            # [head A cols 0:512 | head B cols 512:1024]
            hA, hB = 2 * hp, 2 * hp + 1
            if g == 0:
                pv_acc[(hp, c)] = (
                    ps.tile([D + 8, 512], f32, tag="pv", bufs=2,
                            name=f"pvA{hp}{c}"),
                    ps.tile([D + 8, 512], f32, tag="pv", bufs=2,
                            name=f"pvB{hp}{c}"),
                )
            pvA, pvB = pv_acc[(hp, c)]
            for tt, pt in enumerate((ptA, ptB)):
                t = 2 * g + tt
                nc.tensor.matmul(
                    pvA[:],
                    lhsT=Vb[:, t, hA, :],
                    rhs=pt[:, 0:512],
                    start=t == 0,
                    stop=t == 15,
                )
                nc.tensor.matmul(
                    pvB[:],
                    lhsT=Vb[:, t, hB, :],
                    rhs=pt[:, 512:1024],
                    start=t == 0,
                    stop=t == 15,
                )
            if g == 7:
                cs = slice(c * 512, (c + 1) * 512)
                for pv_, h in ((pvA, hA), (pvB, hB)):
                    po = (h % 2) * D
                    nc.vector.tensor_copy(outT[po:po + D, h // 2, cs], pv_[0:D, :])
                    nc.vector.tensor_tensor(
                        sums[64:64 + HEADS, cs],
                        sums[64:64 + HEADS, cs],
                        pv_[D:D + 8, :],
                        add,
                    )

        # ---- filler queue: (deadline_step, earliest_step, closure) ----
        # deadline: must be emitted by this step (consumer correctness).
        # earliest: must NOT be emitted before this step — for the normalize/
        # output chain it guards in-place accumulation order (sums/outT);
        # for DMA-fed granules it avoids head-of-line PE stalls on DMA.
        fillers = []

        def F(dl, early, fn, *a):
            fillers.append((dl, early, lambda: fn(*a)))

        F(2, 1, emit_k_half, 0, 1)       # keys 512-1023 for g=2,3
        F(4, 3, emit_k_pair, 0, 1)       # keys 1024-2047 for g>=4
        for vg in range(8):              # V for PV(hp0,c0,g=vg) at step vg+PV_LAG
            F(max(0, vg + PV_LAG - 1), min(2 + vg // 2, 6), emit_v_pair, vg)
        F(7, 2, emit_q_half, 0, 1)       # queries 512-1023 for step 8
        for jt in range(1, KT4):
            base = 16 * jt
            F(base - 4, 8, emit_k_pair, jt, 0)
            F(base - 3, 8, emit_k_pair, jt, 1)
            F(base - 2, 8, emit_q_half, jt, 0)
            F(base + 6, 8, emit_q_half, jt, 1)
        F(59, 59, emit_norm_scalar, 0)
        F(60, 60, emit_norm_pair, 0, 0)
        F(60, 60, emit_norm_pair, 0, 1)
        F(61, 61, emit_oproj_pair, 0, 0)
        F(62, 62, emit_oproj_pair, 0, 1)
        # tail (deadline > 63): emitted after the main loop
        F(64, 64, emit_norm_scalar, 1)
        F(64, 64, emit_norm_pair, 1, 0)
        F(64, 64, emit_norm_pair, 1, 1)
        F(65, 65, emit_oproj_pair, 1, 0)
        F(65, 65, emit_oproj_pair, 1, 1)
        fillers.sort(key=lambda x: x[0])

        # ---- warmup: first K/Q granules so attention can start ASAP ----
        emit_k_half(0, 0)
        emit_q_half(0, 0)

        # ---- main interleaved loop ----
        n_early = sum(1 for dl, _, _ in fillers if dl < 64)
        emitted = 0
        pv_pending = []
        step = 0
        for hp in range(KT4):
            for c in range(2):
                for g in range(8):
                    # forced fillers (deadline), then smooth drain (respects
                    # each filler's earliest bound)
                    while fillers and fillers[0][0] <= step:
                        fillers.pop(0)[2]()
                        emitted += 1
                    target = (n_early * (step + 1) + 63) // 64
                    while (fillers and emitted < target and fillers[0][0] < 64
                           and fillers[0][1] <= step):
                        fillers.pop(0)[2]()
                        emitted += 1
                    pt0, pt1 = emit_s_exp(hp, c, g)
                    pv_pending.append((hp, c, g, pt0, pt1))
                    if len(pv_pending) > PV_LAG:
                        emit_pv(*pv_pending.pop(0))
                    step += 1
        # ---- tail: flush PV, then remaining fillers (norm + oproj c1) ----
        while pv_pending:
            emit_pv(*pv_pending.pop(0))
        if debug:
            nc.sync.dma_start(dsums.ap(), sums[64:64 + HEADS, :])
        while fillers:
            fillers.pop(0)[2]()
        if debug:
            nc.sync.dma_start(dQT.ap(), QT[:])
            nc.sync.dma_start(dKT.ap(), KTt[:])
            nc.sync.dma_start(dVb.ap(), Vb[:])
            nc.sync.dma_start(drsum.ap(), rsum[:])
            nc.sync.dma_start(doutT.ap(), outT[:])

    nc.finalize()
    return nc


def _get_program(debug=False):
    key = ("nc", debug)
    if key not in _CACHE:
        _CACHE[key] = _build_program(debug)
    return _CACHE[key]


def _host_inputs(x, Wq, bq, Wk, bk, Wv, bv, Wo, bo):
    import ml_dtypes

    f32 = np.float32
    bf16 = ml_dtypes.bfloat16

    def prep_w(W):
        # [in_feat, out_feat] -> [P, KT4, EMB] bf16 (in_feat = kt*128 + p)
        t = np.asarray(W, f32).T.reshape(KT4, P, EMB).transpose(1, 0, 2)
        return np.ascontiguousarray(t).astype(bf16)

    wq = prep_w(Wq)
    wk = prep_w(Wk)
    wv = prep_w(Wv)
    wo = prep_w(Wo)
    bq2 = np.ascontiguousarray(np.asarray(bq, f32).reshape(KT4, P).T)
    bk2 = np.ascontiguousarray(np.asarray(bk, f32).reshape(KT4, P).T)
    bvr = np.ascontiguousarray(np.tile(np.asarray(bv, f32), (P, 1)))
    bor = np.ascontiguousarray(np.tile(np.asarray(bo, f32), (P, 1)))
    sel = np.zeros((HEADS, KT4, P), f32)
    for et in range(KT4):
        for m in range(P):
            sel[et * 2 + m // D, et, m] = 1.0

    shared = dict(wq=wq, wk=wk, wv=wv, wo=wo, bq2=bq2, bk2=bk2,
                  bvr=bvr, bor=bor, sel=sel)
    x = np.asarray(x, f32)
    in_maps = []
    for core in range(NCORES):
        b, hf = core // 2, core % 2
        xb = x[b]
        # queries first; key order is irrelevant as long as K and V agree
        xr = np.concatenate(
            [xb[hf * HALF:(hf + 1) * HALF], xb[(1 - hf) * HALF:(2 - hf) * HALF]], 0
        )
        # [tokens, feat] -> [4cc, P, KT4, 512]: feat = kt*128+p, token = cc*512+j
        xT = xr.T.reshape(KT4, P, N).transpose(1, 0, 2)  # [P, KT4, N]
        xc = xT.reshape(P, KT4, 4, 512).transpose(2, 0, 1, 3)
        in_maps.append(dict(shared, xc=np.ascontiguousarray(xc).astype(bf16)))
    return in_maps


def kernel(x, Wq, bq, Wk, bk, Wv, bv, Wo, bo, _trace=False, _trace_cores=None,
           _debug=False):
    from concourse.bass_utils import run_bass_kernel_spmd

    nc = _get_program(_debug)
    in_maps = _host_inputs(x, Wq, bq, Wk, bk, Wv, bv, Wo, bo)
    res = run_bass_kernel_spmd(
        nc, in_maps, list(range(NCORES)), trace=_trace,
        trace_cores=_trace_cores,
    )
    out = np.empty((B, N, EMB), np.float32)
    for core in range(NCORES):
        b, hf = core // 2, core % 2
        out[b, hf * HALF:(hf + 1) * HALF] = res.results[core]["y"]
    if _trace:
        _CACHE["last_results"] = res
    return out
